# revision 12
# baseline (speedup 1.0000x reference)
"""Trainium2 Bass kernel for nn_AttentionModule_53223234732422.

Computes: RMSNorm -> QKV projections -> interleaved-pair RoPE on Q,K ->
causal softmax attention (16 heads, head_dim 128) -> output projection.

Sharding (8 NeuronCores, tensor parallel over heads):
  - every core computes the RMSNorm (cheap, avoids an activation collective),
  - each core owns 2 heads: QKV projections with column-sliced weights,
    RoPE, causal attention for those heads,
  - per-head context is AllGathered (2 x 1 MiB fp32 per rank),
  - output projection is split column-wise: each core produces 256 output
    features from the full gathered context.

Host-side preparation (layout only):
  - xs transposed to feature-major [E, S] so contractions land on SBUF
    partitions,
  - norm_w folded into the QKV weights,
  - wq/wk rows permuted per head so RoPE pairs are deinterleaved
    (x0 rows 0..63, x1 rows 64..127); scores are permutation invariant,
  - weights pre-rounded to fp32r (11 mantissa bits, RNE) to match the
    on-device rounding path,
  - cos/sin tables (fp16-arange thetas, like the reference) and the 4
    diagonal causal-mask tiles precomputed.

Dtypes: matmuls run fp32r (full-rate fp32 path; ~1.2e-4 input rounding).
Probabilities and V use bf16 (their quantization averages out in the
softmax-weighted sum). PSUM accumulation is fp32 everywhere. Every tensor a
fp32r matmul consumes is written only by fp32r-typed producers (BIR
verifier requirement).
"""

import sys

sys.path.insert(0, "/opt/trn_rl_repo")

import numpy as np

import concourse.bacc as bacc
import concourse.mybir as mybir
import concourse.tile as tile
from concourse.bass import ds, ts

dt = mybir.dt
AF = mybir.ActivationFunctionType
ALU = mybir.AluOpType

S = 2048
E = 2048
H = 16
D = 128
HALF = D // 2
EPS = 1e-6
THETA = 10000.0
N_CORES = 8
HPC = H // N_CORES  # heads per core
JC = HPC * D  # 256: local q/k/v width
EB = E // N_CORES  # 256: output columns per core
ET = E // 128  # 16 feature tiles
TT = S // 128  # 16 token tiles
NS = S // 512  # 4 token strips
CH = ET // 2  # 8 e-tiles per contraction chunk
INV_SQRT_D = float(1.0 / np.sqrt(np.float32(D)))

F32 = dt.float32
F32R = dt.float32r
BF16 = dt.bfloat16

_NC_CACHE = {}


def _build_nc():
    nc = bacc.Bacc(trn_type="TRN2", num_devices=N_CORES)

    xsT = nc.dram_tensor("xsT", [E, S], F32, kind="ExternalInput")
    wqT = nc.dram_tensor("wqT", [E, JC], F32R, kind="ExternalInput")
    wkT = nc.dram_tensor("wkT", [E, JC], F32R, kind="ExternalInput")
    wvT = nc.dram_tensor("wvT", [E, JC], F32R, kind="ExternalInput")
    woT = nc.dram_tensor("woT", [E, EB], F32R, kind="ExternalInput")
    cosF = nc.dram_tensor("cosF", [D, S], F32, kind="ExternalInput")
    sinF = nc.dram_tensor("sinF", [D, S], F32, kind="ExternalInput")
    masksT = nc.dram_tensor("masks", [NS, 128, 512], F32, kind="ExternalInput")
    out_ext = nc.dram_tensor("out", [S, EB], F32, kind="ExternalOutput")

    rg = [list(range(N_CORES))]

    with tile.TileContext(nc) as tc:
        with (
            tc.tile_pool(name="persist", bufs=1) as pp,
            tc.tile_pool(name="dram", bufs=1, space="DRAM") as dpool,
        ):
            ones_f = pp.tile([128, 1], F32, tag="ones_f")
            ones_r = pp.tile([128, 1], F32R, tag="ones_r")
            ones_b = pp.tile([128, 1], BF16, tag="ones_b")
            eps_sc = pp.tile([1, 1], F32, tag="eps_sc")
            nc.vector.memset(ones_f[:], 1.0)
            nc.vector.tensor_copy(ones_r[:], ones_f[:])
            nc.vector.tensor_copy(ones_b[:], ones_f[:])
            nc.vector.memset(eps_sc[:], EPS)

            # RoPE'd q/k (fp32r; written only by the final rope add) and
            # bf16 token-major v.
            qrope = pp.tile([128, HPC * S], F32R, tag="qrope")
            krope = pp.tile([128, HPC * S], F32R, tag="krope")
            v_sb = pp.tile([128, TT * JC], F32R, tag="v_sb")

            cbounce = [
                dpool.tile([128, S], F32R, tag=f"cb{m}", name=f"cb{m}")
                for m in range(HPC)
            ]
            ag_out = [
                dpool.tile(
                    [N_CORES * 128, S],
                    F32R,
                    addr_space="Shared",
                    tag=f"ag{m}",
                    name=f"ag{m}",
                )
                for m in range(HPC)
            ]

            # ---------------- Phase A: sum-of-squares + rms ----------------
            with tc.tile_pool(name="bcC", bufs=1) as bcp:
                bcastR = bcp.tile([128, S], F32, tag="bcastR")
                with tc.tile_pool(name="rmsp", bufs=1) as rmsp:
                    rms_row = rmsp.tile([1, S], F32, tag="rms_row")
                    with (
                        tc.tile_pool(name="xsA", bufs=4) as xap,
                        tc.tile_pool(name="sqp", bufs=2) as sqp,
                        tc.tile_pool(name="psA", bufs=NS, space="PSUM") as psA,
                    ):
                        ssq_ps = [
                            psA.tile([1, 512], F32, tag="ssq", name="ssq")
                            for _ in range(NS)
                        ]
                        for e in range(ET):
                            xt = xap.tile([128, S], F32, tag="xsA")
                            nc.sync.dma_start(xt[:], xsT[ts(e, 128), :])
                            sq = sqp.tile([128, S], F32R, tag="sq")
                            nc.vector.tensor_mul(sq[:], xt[:], xt[:])
                            for s in range(NS):
                                nc.tensor.matmul(
                                    ssq_ps[s][:],
                                    ones_r[:],
                                    sq[:, ts(s, 512)],
                                    start=(e == 0),
                                    stop=(e == ET - 1),
                                )
                        for s in range(NS):
                            # rms = sqrt(ssq/E + eps)
                            nc.scalar.activation(
                                rms_row[0:1, ts(s, 512)],
                                ssq_ps[s][:],
                                AF.Sqrt,
                                bias=eps_sc[0:1, 0:1],
                                scale=1.0 / E,
                            )
                    nc.vector.reciprocal(rms_row[:], rms_row[:])
                    nc.gpsimd.partition_broadcast(bcastR[:], rms_row[0:1, :])

                # ---------------- Phase C: QKV (2-chunk contraction) -------
                with (
                    tc.tile_pool(name="xraw", bufs=2) as xrp,
                    tc.tile_pool(name="xh", bufs=CH) as xhp,
                    tc.tile_pool(name="wch", bufs=2) as wchp,
                    tc.tile_pool(name="acc", bufs=1) as accp,
                    tc.tile_pool(name="trig", bufs=1) as trigp,
                    tc.tile_pool(name="rsw", bufs=2) as rsp,
                    tc.tile_pool(name="psQK", bufs=2, space="PSUM") as psQK,
                    tc.tile_pool(name="psV", bufs=2, space="PSUM") as psV,
                ):
                    cos_sb = trigp.tile([D, S], F32, tag="cos_sb")
                    sin_sb = trigp.tile([D, S], F32, tag="sin_sb")
                    nc.sync.dma_start(cos_sb[:], cosF[:])
                    nc.sync.dma_start(sin_sb[:], sinF[:])

                    qacc = accp.tile([128, HPC * S], F32, tag="qacc")
                    kacc = accp.tile([128, HPC * S], F32, tag="kacc")

                    for chunk in range(2):
                        # weights for this chunk: [128, CH*JC], e-tile i at
                        # cols i*JC.
                        wtiles = []
                        for wdram in (wqT, wkT, wvT):
                            wc = wchp.tile(
                                [128, CH * JC], F32R, tag="wch",
                                name=f"w{chunk}_{wdram.name}",
                            )
                            src = wdram[ds(chunk * CH * 128, CH * 128), :]
                            nc.sync.dma_start(
                                wc[:].rearrange("p (a j) -> p a j", a=CH),
                                src.rearrange("(a p) j -> p a j", p=128),
                            )
                            wtiles.append(wc)
                        wq_c, wk_c, wv_c = wtiles

                        xh = []
                        for i in range(CH):
                            e = chunk * CH + i
                            xr = xrp.tile([128, S], F32, tag="xraw")
                            nc.sync.dma_start(xr[:], xsT[ts(e, 128), :])
                            xt = xhp.tile([128, S], F32R, tag="xh")
                            nc.vector.tensor_mul(xt[:], xr[:], bcastR[:])
                            xh.append(xt)

                        # q and k projections -> d-major [j, t]
                        for wc, acc, rope_dst in (
                            (wq_c, qacc, qrope),
                            (wk_c, kacc, krope),
                        ):
                            for m in range(HPC):
                                for s in range(NS):
                                    ps = psQK.tile(
                                        [128, 512], F32, tag="qk_ps", name="qk_ps"
                                    )
                                    for i in range(CH):
                                        nc.tensor.matmul(
                                            ps[:],
                                            wc[:, ds(i * JC + m * D, D)],
                                            xh[i][:, ts(s, 512)],
                                            start=(i == 0),
                                            stop=(i == CH - 1),
                                        )
                                    asl = acc[:, ds(m * S + s * 512, 512)]
                                    if chunk == 0:
                                        nc.vector.tensor_copy(asl, ps[:])
                                    else:
                                        nc.vector.scalar_tensor_tensor(
                                            asl, ps[:], 1.0, asl, ALU.mult, ALU.add
                                        )
                                        # RoPE: r = cos*q + sin*swap64(q)
                                        sw = rsp.tile(
                                            [128, 512], F32, tag="rsw", name="rsw"
                                        )
                                        nc.vector.tensor_copy(
                                            sw[0:64, :], asl[64:128, :]
                                        )
                                        nc.vector.tensor_copy(
                                            sw[64:128, :], asl[0:64, :]
                                        )
                                        nc.vector.tensor_mul(
                                            asl, asl, cos_sb[:, ts(s, 512)]
                                        )
                                        nc.vector.tensor_mul(
                                            sw[:], sw[:], sin_sb[:, ts(s, 512)]
                                        )
                                        nc.vector.tensor_tensor(
                                            rope_dst[:, ds(m * S + s * 512, 512)],
                                            asl,
                                            sw[:],
                                            ALU.add,
                                        )

                        # v projection -> token-major bf16 [t, j]
                        for t in range(TT):
                            ps = psV.tile([128, JC], F32, tag="v_ps", name="v_ps")
                            for i in range(CH):
                                nc.tensor.matmul(
                                    ps[:],
                                    xh[i][:, ts(t, 128)],
                                    wv_c[:, ts(i, JC)],
                                    start=(i == 0),
                                    stop=(i == CH - 1),
                                )
                            vsl = v_sb[:, ts(t, JC)]
                            if chunk == 0:
                                nc.vector.tensor_copy(vsl, ps[:])
                            else:
                                nc.vector.scalar_tensor_tensor(
                                    vsl, ps[:], 1.0, vsl, ALU.mult, ALU.add
                                )

            # ---------------- Phase D: attention ----------------
            with (
                tc.tile_pool(name="attn", bufs=1) as apl,
                tc.tile_pool(name="probs", bufs=8) as prp,
                tc.tile_pool(name="bcD", bufs=2) as bdp,
                tc.tile_pool(name="psS", bufs=4, space="PSUM") as psS,
                tc.tile_pool(name="psCtx", bufs=2, space="PSUM") as psC,
                tc.tile_pool(name="psSum", bufs=2, space="PSUM") as psU,
            ):
                ctx_sb = apl.tile([128, HPC * S], F32R, tag="ctx_sb")
                # single lower-triangle mask tile: tri[i, c] = 1 iff i <= c
                tri = apl.tile([128, 128], F32, tag="tri")
                nc.sync.dma_start(tri[:], masksT[0, :, 0:128])

                for m in range(HPC):
                    for s in range(NS):
                        n_tk = 4 * (s + 1)
                        ctx_ps = psC.tile([128, 512], F32, tag="ctx_ps", name="ctx_ps")
                        sum_ps = psU.tile([1, 512], F32, tag="sum_ps", name="sum_ps")
                        for j in range(n_tk):
                            p_rel = j - 4 * s
                            # diagonal blocks only attend to tq_local >= off
                            off = 128 * p_rel if p_rel >= 0 else 0
                            n = 512 - off
                            sc = psS.tile([128, 512], F32, tag="sc", name="sc")
                            nc.tensor.matmul(
                                sc[:, 0:n],
                                krope[:, ds(m * S + j * 128, 128)],
                                qrope[:, ds(m * S + s * 512 + off, n)],
                                start=True,
                                stop=True,
                            )
                            pr = prp.tile([128, 512], F32R, tag="probs", name="pr")
                            if p_rel >= 0:
                                # triangle (first 128 cols of the valid range)
                                et = prp.tile([128, 128], F32, tag="expt", name="et")
                                nc.scalar.activation(
                                    et[:], sc[:, 0:128], AF.Exp, scale=INV_SQRT_D
                                )
                                nc.vector.tensor_mul(pr[:, 0:128], et[:], tri[:])
                                if n > 128:
                                    nc.scalar.activation(
                                        pr[:, 128:n],
                                        sc[:, 128:n],
                                        AF.Exp,
                                        scale=INV_SQRT_D,
                                    )
                            else:
                                nc.scalar.activation(
                                    pr[:, 0:n], sc[:, 0:n], AF.Exp, scale=INV_SQRT_D
                                )
                            nc.tensor.matmul(
                                ctx_ps[:, ds(off, n)],
                                v_sb[:, ds(j * JC + m * D, D)],
                                pr[:, 0:n],
                                start=(j == 0),
                                stop=(j == n_tk - 1),
                            )
                            nc.tensor.matmul(
                                sum_ps[0:1, ds(off, n)],
                                ones_r[:],
                                pr[:, 0:n],
                                start=(j == 0),
                                stop=(j == n_tk - 1),
                            )
                        rr = bdp.tile([1, 512], F32, tag="recip", name="rr")
                        nc.vector.reciprocal(rr[:], sum_ps[:])
                        bc = bdp.tile([128, 512], F32, tag="bcD", name="bc")
                        nc.gpsimd.partition_broadcast(bc[:], rr[0:1, :])
                        nc.vector.tensor_mul(
                            ctx_sb[:, ds(m * S + s * 512, 512)], ctx_ps[:], bc[:]
                        )
                    nc.sync.dma_start(cbounce[m][:], ctx_sb[:, ts(m, S)])
                    nc.gpsimd.collective_compute(
                        "AllGather",
                        ALU.bypass,
                        replica_groups=rg,
                        ins=[cbounce[m].opt()],
                        outs=[ag_out[m].opt()],
                    )

            # ---------------- Phase E: output projection ----------------
            with (
                tc.tile_pool(name="ck", bufs=ET) as ckp,
                tc.tile_pool(name="wo", bufs=1) as wop,
                tc.tile_pool(name="ob", bufs=2) as obp,
                tc.tile_pool(name="psW", bufs=3, space="PSUM") as psW,
            ):
                woT_sb = wop.tile([128, ET * EB], F32R, tag="woT_sb")
                nc.sync.dma_start(
                    woT_sb[:].rearrange("p (a j) -> p a j", a=ET),
                    woT[:, :].rearrange("(a p) j -> p a j", p=128),
                )
                ctxk = []
                for kb in range(ET):
                    ct = ckp.tile([128, S], F32R, tag="ck", name=f"ck{kb}")
                    src = ag_out[0] if kb < CH else ag_out[1]
                    nc.sync.dma_start(ct[:], src[ts(kb % CH, 128), :])
                    ctxk.append(ct)
                for t in range(TT):
                    ps = psW.tile([128, EB], F32, tag="wo_ps", name="wo_ps")
                    for kb in range(ET):
                        nc.tensor.matmul(
                            ps[:],
                            ctxk[kb][:, ts(t, 128)],
                            woT_sb[:, ts(kb, EB)],
                            start=(kb == 0),
                            stop=(kb == ET - 1),
                        )
                    ob = obp.tile([128, EB], F32, tag="ob", name="ob")
                    nc.vector.tensor_copy(ob[:], ps[:])
                    nc.sync.dma_start(out_ext[ts(t, 128), :], ob[:])

    nc.compile()
    return nc


def get_nc():
    if "nc" not in _NC_CACHE:
        _NC_CACHE["nc"] = _build_nc()
    return _NC_CACHE["nc"]


def _round_f32r(a):
    """Round fp32 to fp32r (11 explicit mantissa bits) with RNE."""
    u = np.ascontiguousarray(a, dtype=np.float32).view(np.uint32).copy()
    round_bit = (u >> 12) & 1
    u += 0x7FF + round_bit
    u &= np.uint32(0xFFFFF000)
    return u.view(np.float32)


def _rope_tables():
    """thetas with the reference's fp16-arange quirk, then f32 cos/sin."""
    try:
        # Same ops/dtypes as the reference, on the default jax device, so
        # the fp16 pow rounds identically to the reference run in this env.
        import jax.numpy as jnp

        th = (
            THETA ** (-jnp.arange(HALF, dtype=jnp.float16) / HALF)
        ).astype(jnp.float32)
        thetas = np.asarray(th)
    except Exception:
        ar = np.arange(HALF, dtype=np.float16)
        y = -ar / np.float16(HALF)
        thetas = (np.float16(THETA) ** y).astype(np.float32)
    m = np.arange(S, dtype=np.float32)
    ang = m[:, None] * thetas[None, :]  # [S, 64] f32
    cos = np.ascontiguousarray(np.cos(ang).astype(np.float32).T)  # [64, S]
    sin = np.ascontiguousarray(np.sin(ang).astype(np.float32).T)
    cosF = np.concatenate([cos, cos], axis=0)  # [128, S]
    sinF = np.concatenate([-sin, sin], axis=0)
    return np.ascontiguousarray(cosF), np.ascontiguousarray(sinF)


def _host_prep(xs, norm_w, wq, wk, wv, wo):
    xs = np.asarray(xs, dtype=np.float32)
    norm_w = np.asarray(norm_w, dtype=np.float32)
    wq = np.asarray(wq, dtype=np.float32)
    wk = np.asarray(wk, dtype=np.float32)
    wv = np.asarray(wv, dtype=np.float32)
    wo = np.asarray(wo, dtype=np.float32)

    xsT = np.ascontiguousarray(xs.T)
    cosF, sinF = _rope_tables()

    i = np.arange(128)[:, None]
    tq = np.arange(512)[None, :]
    masks = np.stack(
        [((128 * p + i) <= tq).astype(np.float32) for p in range(NS)]
    )

    perm = np.concatenate([np.arange(0, D, 2), np.arange(1, D, 2)])
    wq_n = wq * norm_w[None, :]
    wk_n = wk * norm_w[None, :]
    wv_n = wv * norm_w[None, :]
    f_order = np.concatenate(
        [np.arange(h * D, (h + 1) * D) for h in range(0, H, 2)]
        + [np.arange(h * D, (h + 1) * D) for h in range(1, H, 2)]
    )

    in_maps = []
    for c in range(N_CORES):
        heads = (2 * c, 2 * c + 1)
        rows_qk = np.concatenate([h * D + perm for h in heads])
        rows_v = np.concatenate([np.arange(h * D, (h + 1) * D) for h in heads])
        in_maps.append(
            {
                "xsT": xsT,
                "wqT": _round_f32r(np.ascontiguousarray(wq_n[rows_qk].T)),
                "wkT": _round_f32r(np.ascontiguousarray(wk_n[rows_qk].T)),
                "wvT": _round_f32r(np.ascontiguousarray(wv_n[rows_v].T)),
                "woT": _round_f32r(
                    np.ascontiguousarray(wo[c * EB : (c + 1) * EB, :].T[f_order, :])
                ),
                "cosF": cosF,
                "sinF": sinF,
                "masks": masks,
            }
        )
    return in_maps


def kernel(xs, norm_w, wq, wk, wv, wo):
    from concourse.bass_utils import run_bass_kernel_spmd

    nc = get_nc()
    in_maps = _host_prep(xs, norm_w, wq, wk, wv, wo)
    res = run_bass_kernel_spmd(nc, in_maps, list(range(N_CORES)))
    out = np.concatenate([res.results[c]["out"] for c in range(N_CORES)], axis=1)
    return out.astype(np.float32)
